# revision 20
# baseline (speedup 1.0000x reference)
"""Single-head causal self-attention on 8 trn2 NeuronCores.

B=16, T=4096, D=64 fp32. Data-parallel over batch: 2 batches per core.
Per core/batch: x -> xT (PE transpose), QT/KT (dup'd weight proj), V1=[V|1]
natural layout; scores computed transposed (keys on partitions) in f32r,
exp on ScalarE with fused 1/sqrt(D) scale, causal via chunk skipping +
N-restriction + affine_select on diagonal blocks; PV matmul accumulates
[Y^T; denom] in PSUM via ones-column; normalization happens on host.
"""
import os
import sys

os.environ.setdefault("MYCRO_LOCAL_CACHE", "1")
sys.path.insert(0, "/opt/trn_rl_repo")

import numpy as np

import concourse.bass as bass
import concourse.tile as tile
from concourse import bacc, mybir
from concourse.bass_utils import run_bass_kernel_spmd

F32 = mybir.dt.float32
F32R = mybir.dt.float32r
F16 = mybir.dt.float16

N_CORES = 8
B_LOC = 2          # batches per core
T = 4096
D = 64
NQ = 8             # q-superblocks of 512 per batch
QB = 512           # q-superblock width
KB = 128           # key chunk (PSUM partition dim of scores)
NCHUNK = T // KB   # 32 key chunks per batch


def _build():
    nc = bacc.Bacc(None)

    xt_d = nc.declare_dram_parameter("xt16", [B_LOC, 128, T], F16, isOutput=False)
    wq2_d = nc.declare_dram_parameter("wq2", [128, 128], F32, isOutput=False)
    wk2_d = nc.declare_dram_parameter("wk2", [128, 128], F32, isOutput=False)
    wv_d = nc.declare_dram_parameter("wv", [128, D], F32, isOutput=False)
    id_d = nc.declare_dram_parameter("ident", [128, 128], F32, isOutput=False)
    yt1_d = nc.declare_dram_parameter("yt1", [B_LOC, NQ, D + 1, QB], F32, isOutput=True)

    with tile.TileContext(nc) as tc:
        with (
            tc.tile_pool(name="consts", bufs=1) as consts,
            tc.tile_pool(name="xt", bufs=2) as xt_p,
            tc.tile_pool(name="qt", bufs=2) as qt_p,
            tc.tile_pool(name="kt", bufs=2) as kt_p,
            tc.tile_pool(name="v1", bufs=2) as v1_p,
            tc.tile_pool(name="pt", bufs=3) as pt_p,
            tc.tile_pool(name="scratch", bufs=2) as scratch_p,
            tc.tile_pool(name="stps", bufs=2, space="PSUM") as st_ps,
            tc.tile_pool(name="ytps", bufs=2, space="PSUM") as yt_ps,
        ):
            # ---- constants ----
            ident = consts.tile([128, 128], F16, tag="ident")
            nc.gpsimd.dma_start(out=ident, in_=id_d[:, :])
            wq2 = consts.tile([128, 128], F16, tag="wq2")
            nc.gpsimd.dma_start(out=wq2, in_=wq2_d[:, :])
            wk2 = consts.tile([128, 128], F16, tag="wk2")
            nc.gpsimd.dma_start(out=wk2, in_=wk2_d[:, :])
            wv = consts.tile([128, D], F16, tag="wv")
            nc.gpsimd.dma_start(out=wv, in_=wv_d[:, :])

            nbias = consts.tile([128, 1], F32, tag="nbias")
            nc.vector.memset(nbias, -8.0)

            # ---- warmups: ACT table load + PE HAM ramp ----
            wsc = scratch_p.tile([128, 128], F32, tag="wexp")
            nc.scalar.activation(out=wsc, in_=wq2, func=mybir.ActivationFunctionType.Exp, scale=0.01)
            for _ in range(8):
                wps = yt_ps.tile([128, 128], F32, tag="yt1", name="wps")
                nc.tensor.matmul(out=wps, lhsT=ident, rhs=ident, start=True, stop=True)

            state = {}

            def make_prologue(b):
                xt = xt_p.tile([128, T], F16, tag="xt", name="xt")
                nc.sync.dma_start(out=xt, in_=xt_d[b])
                # Q/K projections (weights duplicated -> output rows 0:64 and 64:128)
                qt = qt_p.tile([128, T], F16, tag="qt", name="qt")
                kt = kt_p.tile([128, T], F16, tag="kt", name="kt")
                v1 = v1_p.tile([128, NCHUNK, D + 1], F16, tag="v1", name="v1")
                nc.vector.memset(v1[:, :, D : D + 1], 1.0)
                state[b] = (qt, kt, v1)

                def proj_qk(j):
                    pq = yt_ps.tile([128, QB], F32, tag="yt1", name="pq")
                    hq = 64 * (j % 2)
                    nc.tensor.matmul(out=pq, lhsT=wq2[hq : hq + 64, :], rhs=xt[hq : hq + 64, QB * j : QB * (j + 1)], start=True, stop=True)
                    nc.vector.tensor_copy(out=qt[:, QB * j : QB * (j + 1)], in_=pq)
                    pk = yt_ps.tile([128, QB], F32, tag="yt1", name="pk")
                    nc.tensor.matmul(out=pk, lhsT=wk2[64 - hq : 128 - hq, :], rhs=xt[64 - hq : 128 - hq, QB * j : QB * (j + 1)], start=True, stop=True)
                    nc.vector.tensor_copy(out=kt[:, QB * j : QB * (j + 1)], in_=pk)

                def proj_v(g):
                    pvp = yt_ps.tile([128, QB], F32, tag="yt1", name="pvp")
                    for k in range(8):
                        t = 8 * g + k
                        nc.tensor.matmul(
                            out=pvp[:, D * k : D * (k + 1)],
                            lhsT=xt[0:64, 128 * t : 128 * (t + 1)],
                            rhs=wv[0:64, :],
                            start=True,
                            stop=True,
                        )
                    nc.vector.tensor_copy(
                        out=v1[:, 8 * g : 8 * (g + 1), 0:D],
                        in_=pvp.rearrange("p (k c) -> p k c", c=D),
                    )

                return proj_qk, proj_v

            TPC = 3  # chunks per ST/PT tile (ACT granularity = 1536 cols)

            def main_superblock(b, m):
                qt, kt, v1 = state[b]
                nch = 4 * m + 4
                ntiles = (nch + TPC - 1) // TPC
                yt1 = yt_ps.tile([128, QB], F32, tag="yt1")
                st_tiles = []
                pt_tiles = []

                def emit_st(c):
                    ti, slot = divmod(c, TPC)
                    if slot == 0:
                        st_tiles.append(st_ps.tile([128, QB * TPC], F32, tag="st", name="st_t"))
                    st_t = st_tiles[ti]
                    j = c - 4 * m
                    qoff = 128 * j if j >= 0 else 0
                    half = 64 * (c % 2)
                    nc.tensor.matmul(
                        out=st_t[:, QB * slot + qoff : QB * (slot + 1)],
                        lhsT=kt[half : half + 64, KB * c : KB * (c + 1)],
                        rhs=qt[half : half + 64, QB * m + qoff : QB * (m + 1)],
                        start=True,
                        stop=True,
                    )

                def emit_act_pv(ti):
                    st_t = st_tiles[ti]
                    c0 = TPC * ti
                    c1 = min(c0 + TPC, nch)
                    width = QB * (c1 - c0)
                    pt = pt_p.tile([128, QB * TPC], F16, tag="pt", name="pt")
                    pt_tiles.append(pt)
                    garbage = sum(
                        128 * (c - 4 * m) for c in range(c0, c1) if c - 4 * m > 0
                    )
                    if garbage >= 384:
                        # skip exp over never-read columns of diagonal chunks
                        for c in range(c0, c1):
                            j = c - 4 * m
                            qoff = 128 * j if j >= 0 else 0
                            slot = c - c0
                            nc.scalar.activation(
                                out=pt[:, QB * slot + qoff : QB * (slot + 1)],
                                in_=st_t[:, QB * slot + qoff : QB * (slot + 1)],
                                func=mybir.ActivationFunctionType.Exp,
                                bias=nbias,
                                scale=0.125,
                            )
                    else:
                        nc.scalar.activation(
                            out=pt[:, :width],
                            in_=st_t[:, :width],
                            func=mybir.ActivationFunctionType.Exp,
                            bias=nbias,
                            scale=0.125,
                        )
                    for c in range(c0, c1):
                        j = c - 4 * m
                        if j >= 0:
                            slot = c - c0
                            sub = pt[:, QB * slot + 128 * j : QB * slot + 128 * (j + 1)]
                            nc.gpsimd.affine_select(
                                out=sub,
                                in_=sub,
                                compare_op=mybir.AluOpType.is_ge,
                                fill=0.0,
                                base=0,
                                pattern=[[1, 128]],
                                channel_multiplier=-1,
                            )
                    for c in range(c0, c1):
                        j = c - 4 * m
                        qoff = 128 * j if j >= 0 else 0
                        slot = c - c0
                        nc.tensor.matmul(
                            out=yt1[0 : D + 1, qoff:QB],
                            lhsT=v1[:, c, :],
                            rhs=pt[:, QB * slot + qoff : QB * (slot + 1)],
                            start=(c == 0),
                            stop=(c == nch - 1),
                            skip_group_check=True,
                        )

                for c in range(min(TPC, nch)):
                    emit_st(c)
                for ti in range(1, ntiles):
                    for c in range(TPC * ti, min(TPC * (ti + 1), nch)):
                        emit_st(c)
                    emit_act_pv(ti - 1)
                emit_act_pv(ntiles - 1)
                ytsb = scratch_p.tile([D + 1, QB], F32, tag="ytsb", name="ytsb")
                nc.vector.tensor_copy(out=ytsb, in_=yt1[0 : D + 1, :])
                nc.sync.dma_start(out=yt1_d[b, m, :, :], in_=ytsb)

            # emission: projections interleaved just-in-time between
            # superblocks so ACT never starves; b1 prologue spread over
            # b0's large tail superblocks.
            pq0, pv0 = make_prologue(0)
            pq0(0); pv0(0)
            main_superblock(0, 0)
            pq0(1); main_superblock(0, 1)
            pq0(2); pv0(1); main_superblock(0, 2)
            pq0(3); main_superblock(0, 3)
            pq0(4); pv0(2); main_superblock(0, 4)
            pq1, pv1 = make_prologue(1)
            pq0(5); pv0(3); main_superblock(0, 5)
            pq0(6); pq1(0); pv1(0); main_superblock(0, 6)
            pq0(7); pq1(1); pq1(2); pv1(1); main_superblock(0, 7)
            pq1(3); pq1(4); pv1(2); main_superblock(1, 0)
            pq1(5); pq1(6); pv1(3); main_superblock(1, 1)
            pq1(7); main_superblock(1, 2)
            for m in range(3, NQ):
                main_superblock(1, m)

    nc.finalize()
    return nc


_NC = None


def _get_nc():
    global _NC
    if _NC is None:
        _NC = _build()
    return _NC


def _run(x, Wk, Wq, Wv, trace=False):
    x = np.ascontiguousarray(np.asarray(x, dtype=np.float32))
    Wk = np.asarray(Wk, dtype=np.float32)
    Wq = np.asarray(Wq, dtype=np.float32)
    Wv = np.asarray(Wv, dtype=np.float32)
    B = x.shape[0]
    assert B == N_CORES * B_LOC and x.shape[1] == T and x.shape[2] == D

    wq2 = np.concatenate([Wq.T, Wq.T], axis=1)
    wq2 = np.ascontiguousarray(np.concatenate([wq2, wq2], axis=0))
    wk2 = np.concatenate([Wk.T, Wk.T], axis=1)
    wk2 = np.ascontiguousarray(np.concatenate([wk2, wk2], axis=0))
    wv = np.ascontiguousarray(np.concatenate([Wv.T, Wv.T], axis=0))
    ident = np.eye(128, dtype=np.float32)

    xt16 = x.astype(np.float16).transpose(0, 2, 1)
    xt16 = np.ascontiguousarray(np.concatenate([xt16, xt16], axis=1))
    in_maps = []
    for c in range(N_CORES):
        in_maps.append(
            {
                "xt16": np.ascontiguousarray(xt16[B_LOC * c : B_LOC * (c + 1)]),
                "wq2": wq2,
                "wk2": wk2,
                "wv": wv,
                "ident": ident,
            }
        )

    nc = _get_nc()
    res = run_bass_kernel_spmd(nc, in_maps, core_ids=list(range(N_CORES)), trace=trace)

    y = np.empty((B, T, D), dtype=np.float32)
    for c in range(N_CORES):
        yt1 = res.results[c]["yt1"]  # [B_LOC, NQ, 65, 512]
        num = yt1[:, :, :D, :]
        den = yt1[:, :, D : D + 1, :]
        yb = (num / den).transpose(0, 1, 3, 2).reshape(B_LOC, T, D)
        y[B_LOC * c : B_LOC * (c + 1)] = yb
    return y, res


def kernel(x, Wk, Wq, Wv):
    y, _ = _run(x, Wk, Wq, Wv, trace=False)
    return y


# revision 21
# speedup vs baseline: 1.1902x; 1.1902x over previous
"""Single-head causal self-attention on 8 trn2 NeuronCores.

B=16, T=4096, D=64 fp32. Data-parallel over batch: 2 batches per core.
Per core/batch: x -> xT (PE transpose), QT/KT (dup'd weight proj), V1=[V|1]
natural layout; scores computed transposed (keys on partitions) in f32r,
exp on ScalarE with fused 1/sqrt(D) scale, causal via chunk skipping +
N-restriction + affine_select on diagonal blocks; PV matmul accumulates
[Y^T; denom] in PSUM via ones-column; normalization happens on host.
"""
import os
import sys

os.environ.setdefault("MYCRO_LOCAL_CACHE", "1")
sys.path.insert(0, "/opt/trn_rl_repo")

import numpy as np

import concourse.bass as bass
import concourse.tile as tile
from concourse import bacc, mybir
from concourse.bass_utils import run_bass_kernel_spmd

F32 = mybir.dt.float32
F32R = mybir.dt.float32r
F16 = mybir.dt.float16

N_CORES = 8
B_LOC = 2          # batches per core
T = 4096
D = 64
NQ = 8             # q-superblocks of 512 per batch
QB = 512           # q-superblock width
KB = 128           # key chunk (PSUM partition dim of scores)
NCHUNK = T // KB   # 32 key chunks per batch


def _build():
    nc = bacc.Bacc(None)

    xt_d = nc.declare_dram_parameter("xt16", [B_LOC, 128, T], F16, isOutput=False)
    wq2_d = nc.declare_dram_parameter("wq2", [128, 128], F32, isOutput=False)
    wk2_d = nc.declare_dram_parameter("wk2", [128, 128], F32, isOutput=False)
    wv_d = nc.declare_dram_parameter("wv", [128, D], F32, isOutput=False)
    id_d = nc.declare_dram_parameter("ident", [128, 128], F32, isOutput=False)
    yt1_d = nc.declare_dram_parameter("yt1", [B_LOC, NQ, D + 1, QB], F32, isOutput=True)

    with tile.TileContext(nc) as tc:
        with (
            tc.tile_pool(name="consts", bufs=1) as consts,
            tc.tile_pool(name="xt", bufs=2) as xt_p,
            tc.tile_pool(name="qt", bufs=2) as qt_p,
            tc.tile_pool(name="kt", bufs=2) as kt_p,
            tc.tile_pool(name="v1", bufs=2) as v1_p,
            tc.tile_pool(name="pt", bufs=3) as pt_p,
            tc.tile_pool(name="scratch", bufs=2) as scratch_p,
            tc.tile_pool(name="stps", bufs=2, space="PSUM") as st_ps,
            tc.tile_pool(name="ytps", bufs=2, space="PSUM") as yt_ps,
            tc.tile_pool(name="prps", bufs=2, space="PSUM") as pr_ps,
        ):
            # ---- constants ----
            ident = consts.tile([128, 128], F16, tag="ident")
            nc.gpsimd.dma_start(out=ident, in_=id_d[:, :])
            wq2 = consts.tile([128, 128], F16, tag="wq2")
            nc.gpsimd.dma_start(out=wq2, in_=wq2_d[:, :])
            wk2 = consts.tile([128, 128], F16, tag="wk2")
            nc.gpsimd.dma_start(out=wk2, in_=wk2_d[:, :])
            wv = consts.tile([128, D], F16, tag="wv")
            nc.gpsimd.dma_start(out=wv, in_=wv_d[:, :])

            nbias = consts.tile([128, 1], F32, tag="nbias")
            nc.vector.memset(nbias, -8.0)

            # ---- warmups: ACT table load + PE HAM ramp ----
            wsc = scratch_p.tile([128, 128], F32, tag="wexp")
            nc.scalar.activation(out=wsc, in_=wq2, func=mybir.ActivationFunctionType.Exp, scale=0.01)
            for _ in range(8):
                wps = pr_ps.tile([128, 128], F32, tag="pro", name="wps")
                nc.tensor.matmul(out=wps, lhsT=ident, rhs=ident, start=True, stop=True)

            state = {}

            def make_prologue(b):
                xt = xt_p.tile([128, T], F16, tag="xt", name="xt")
                nc.sync.dma_start(out=xt, in_=xt_d[b])
                # Q/K projections (weights duplicated -> output rows 0:64 and 64:128)
                qt = qt_p.tile([128, T], F16, tag="qt", name="qt")
                kt = kt_p.tile([128, T], F16, tag="kt", name="kt")
                v1 = v1_p.tile([128, NCHUNK, D + 1], F16, tag="v1", name="v1")
                nc.vector.memset(v1[:, :, D : D + 1], 1.0)
                state[b] = (qt, kt, v1)

                def proj_qk(j):
                    pq = pr_ps.tile([128, QB], F32, tag="pro", name="pq")
                    hq = 64 * (j % 2)
                    nc.tensor.matmul(out=pq, lhsT=wq2[hq : hq + 64, :], rhs=xt[hq : hq + 64, QB * j : QB * (j + 1)], start=True, stop=True)
                    nc.vector.tensor_copy(out=qt[:, QB * j : QB * (j + 1)], in_=pq)
                    pk = pr_ps.tile([128, QB], F32, tag="pro", name="pk")
                    nc.tensor.matmul(out=pk, lhsT=wk2[64 - hq : 128 - hq, :], rhs=xt[64 - hq : 128 - hq, QB * j : QB * (j + 1)], start=True, stop=True)
                    nc.vector.tensor_copy(out=kt[:, QB * j : QB * (j + 1)], in_=pk)

                def proj_v(g):
                    pvp = pr_ps.tile([128, QB], F32, tag="pro", name="pvp")
                    for k in range(8):
                        t = 8 * g + k
                        nc.tensor.matmul(
                            out=pvp[:, D * k : D * (k + 1)],
                            lhsT=xt[0:64, 128 * t : 128 * (t + 1)],
                            rhs=wv[0:64, :],
                            start=True,
                            stop=True,
                        )
                    nc.vector.tensor_copy(
                        out=v1[:, 8 * g : 8 * (g + 1), 0:D],
                        in_=pvp.rearrange("p (k c) -> p k c", c=D),
                    )

                return proj_qk, proj_v

            TPC = 2  # chunks per ST/PT tile (ACT granularity = 1024 cols)

            def main_superblock(b, m):
                qt, kt, v1 = state[b]
                nch = 4 * m + 4
                ntiles = (nch + TPC - 1) // TPC
                yt1 = yt_ps.tile([128, QB], F32, tag="yt1")
                st_tiles = []
                pt_tiles = []

                def emit_st(c):
                    ti, slot = divmod(c, TPC)
                    if slot == 0:
                        st_tiles.append(st_ps.tile([128, QB * TPC], F32, tag="st", name="st_t"))
                    st_t = st_tiles[ti]
                    j = c - 4 * m
                    qoff = 128 * j if j >= 0 else 0
                    half = 64 * (c % 2)
                    nc.tensor.matmul(
                        out=st_t[:, QB * slot + qoff : QB * (slot + 1)],
                        lhsT=kt[half : half + 64, KB * c : KB * (c + 1)],
                        rhs=qt[half : half + 64, QB * m + qoff : QB * (m + 1)],
                        start=True,
                        stop=True,
                    )

                def emit_act_pv(ti):
                    st_t = st_tiles[ti]
                    c0 = TPC * ti
                    c1 = min(c0 + TPC, nch)
                    width = QB * (c1 - c0)
                    pt = pt_p.tile([128, QB * TPC], F16, tag="pt", name="pt")
                    pt_tiles.append(pt)
                    garbage = sum(
                        128 * (c - 4 * m) for c in range(c0, c1) if c - 4 * m > 0
                    )
                    if garbage >= 384:
                        # skip exp over never-read columns of diagonal chunks
                        for c in range(c0, c1):
                            j = c - 4 * m
                            qoff = 128 * j if j >= 0 else 0
                            slot = c - c0
                            nc.scalar.activation(
                                out=pt[:, QB * slot + qoff : QB * (slot + 1)],
                                in_=st_t[:, QB * slot + qoff : QB * (slot + 1)],
                                func=mybir.ActivationFunctionType.Exp,
                                bias=nbias,
                                scale=0.125,
                            )
                    else:
                        nc.scalar.activation(
                            out=pt[:, :width],
                            in_=st_t[:, :width],
                            func=mybir.ActivationFunctionType.Exp,
                            bias=nbias,
                            scale=0.125,
                        )
                    for c in range(c0, c1):
                        j = c - 4 * m
                        if j >= 0:
                            slot = c - c0
                            sub = pt[:, QB * slot + 128 * j : QB * slot + 128 * (j + 1)]
                            nc.gpsimd.affine_select(
                                out=sub,
                                in_=sub,
                                compare_op=mybir.AluOpType.is_ge,
                                fill=0.0,
                                base=0,
                                pattern=[[1, 128]],
                                channel_multiplier=-1,
                            )
                    for c in range(c0, c1):
                        j = c - 4 * m
                        qoff = 128 * j if j >= 0 else 0
                        slot = c - c0
                        nc.tensor.matmul(
                            out=yt1[0 : D + 1, qoff:QB],
                            lhsT=v1[:, c, :],
                            rhs=pt[:, QB * slot + qoff : QB * (slot + 1)],
                            start=(c == 0),
                            stop=(c == nch - 1),
                            skip_group_check=True,
                        )

                for c in range(min(TPC, nch)):
                    emit_st(c)
                for ti in range(1, ntiles):
                    for c in range(TPC * ti, min(TPC * (ti + 1), nch)):
                        emit_st(c)
                    emit_act_pv(ti - 1)
                emit_act_pv(ntiles - 1)
                ytsb = scratch_p.tile([D + 1, QB], F32, tag="ytsb", name="ytsb")
                nc.vector.tensor_copy(out=ytsb, in_=yt1[0 : D + 1, :])
                nc.sync.dma_start(out=yt1_d[b, m, :, :], in_=ytsb)

            # emission: projections interleaved just-in-time between
            # superblocks so ACT never starves; b1 prologue spread over
            # b0's large tail superblocks.
            pq0, pv0 = make_prologue(0)
            pq0(0); pv0(0)
            main_superblock(0, 0)
            pq0(1); main_superblock(0, 1)
            pq0(2); pv0(1); main_superblock(0, 2)
            pq0(3); main_superblock(0, 3)
            pq0(4); pv0(2); main_superblock(0, 4)
            pq1, pv1 = make_prologue(1)
            pq0(5); pv0(3); main_superblock(0, 5)
            pq0(6); pq1(0); pv1(0); main_superblock(0, 6)
            pq0(7); pq1(1); pq1(2); pv1(1); main_superblock(0, 7)
            pq1(3); pq1(4); pv1(2); main_superblock(1, 0)
            pq1(5); pq1(6); pv1(3); main_superblock(1, 1)
            pq1(7); main_superblock(1, 2)
            for m in range(3, NQ):
                main_superblock(1, m)

    nc.finalize()
    return nc


_NC = None


def _get_nc():
    global _NC
    if _NC is None:
        _NC = _build()
    return _NC


def _run(x, Wk, Wq, Wv, trace=False):
    x = np.ascontiguousarray(np.asarray(x, dtype=np.float32))
    Wk = np.asarray(Wk, dtype=np.float32)
    Wq = np.asarray(Wq, dtype=np.float32)
    Wv = np.asarray(Wv, dtype=np.float32)
    B = x.shape[0]
    assert B == N_CORES * B_LOC and x.shape[1] == T and x.shape[2] == D

    wq2 = np.concatenate([Wq.T, Wq.T], axis=1)
    wq2 = np.ascontiguousarray(np.concatenate([wq2, wq2], axis=0))
    wk2 = np.concatenate([Wk.T, Wk.T], axis=1)
    wk2 = np.ascontiguousarray(np.concatenate([wk2, wk2], axis=0))
    wv = np.ascontiguousarray(np.concatenate([Wv.T, Wv.T], axis=0))
    ident = np.eye(128, dtype=np.float32)

    xt16 = x.astype(np.float16).transpose(0, 2, 1)
    xt16 = np.ascontiguousarray(np.concatenate([xt16, xt16], axis=1))
    in_maps = []
    for c in range(N_CORES):
        in_maps.append(
            {
                "xt16": np.ascontiguousarray(xt16[B_LOC * c : B_LOC * (c + 1)]),
                "wq2": wq2,
                "wk2": wk2,
                "wv": wv,
                "ident": ident,
            }
        )

    nc = _get_nc()
    res = run_bass_kernel_spmd(nc, in_maps, core_ids=list(range(N_CORES)), trace=trace)

    y = np.empty((B, T, D), dtype=np.float32)
    for c in range(N_CORES):
        yt1 = res.results[c]["yt1"]  # [B_LOC, NQ, 65, 512]
        num = yt1[:, :, :D, :]
        den = yt1[:, :, D : D + 1, :]
        yb = (num / den).transpose(0, 1, 3, 2).reshape(B_LOC, T, D)
        y[B_LOC * c : B_LOC * (c + 1)] = yb
    return y, res


def kernel(x, Wk, Wq, Wv):
    y, _ = _run(x, Wk, Wq, Wv, trace=False)
    return y


# revision 22
# speedup vs baseline: 1.2058x; 1.0131x over previous
"""Single-head causal self-attention on 8 trn2 NeuronCores.

B=16, T=4096, D=64 fp32. Data-parallel over batch: 2 batches per core.
Per core/batch: x -> xT (PE transpose), QT/KT (dup'd weight proj), V1=[V|1]
natural layout; scores computed transposed (keys on partitions) in f32r,
exp on ScalarE with fused 1/sqrt(D) scale, causal via chunk skipping +
N-restriction + affine_select on diagonal blocks; PV matmul accumulates
[Y^T; denom] in PSUM via ones-column; normalization happens on host.
"""
import os
import sys

os.environ.setdefault("MYCRO_LOCAL_CACHE", "1")
sys.path.insert(0, "/opt/trn_rl_repo")

import numpy as np

import concourse.bass as bass
import concourse.tile as tile
from concourse import bacc, mybir
from concourse.bass_utils import run_bass_kernel_spmd

F32 = mybir.dt.float32
F32R = mybir.dt.float32r
F16 = mybir.dt.float16

N_CORES = 8
B_LOC = 2          # batches per core
T = 4096
D = 64
NQ = 8             # q-superblocks of 512 per batch
QB = 512           # q-superblock width
KB = 128           # key chunk (PSUM partition dim of scores)
NCHUNK = T // KB   # 32 key chunks per batch


def _build():
    nc = bacc.Bacc(None)

    xt_d = nc.declare_dram_parameter("xt16", [B_LOC, 128, T], F16, isOutput=False)
    wq2_d = nc.declare_dram_parameter("wq2", [128, 128], F32, isOutput=False)
    wk2_d = nc.declare_dram_parameter("wk2", [128, 128], F32, isOutput=False)
    wv_d = nc.declare_dram_parameter("wv", [128, D], F32, isOutput=False)
    id_d = nc.declare_dram_parameter("ident", [128, 128], F32, isOutput=False)
    yt1_d = nc.declare_dram_parameter("yt1", [B_LOC, NQ, D + 1, QB], F32, isOutput=True)

    with tile.TileContext(nc) as tc:
        with (
            tc.tile_pool(name="consts", bufs=1) as consts,
            tc.tile_pool(name="xt", bufs=2) as xt_p,
            tc.tile_pool(name="qt", bufs=2) as qt_p,
            tc.tile_pool(name="kt", bufs=2) as kt_p,
            tc.tile_pool(name="v1", bufs=2) as v1_p,
            tc.tile_pool(name="pt", bufs=3) as pt_p,
            tc.tile_pool(name="scratch", bufs=2) as scratch_p,
            tc.tile_pool(name="stps", bufs=2, space="PSUM") as st_ps,
            tc.tile_pool(name="ytps", bufs=2, space="PSUM") as yt_ps,
            tc.tile_pool(name="prps", bufs=2, space="PSUM") as pr_ps,
        ):
            # ---- constants ----
            ident = consts.tile([128, 128], F16, tag="ident")
            nc.gpsimd.dma_start(out=ident, in_=id_d[:, :])
            wq2 = consts.tile([128, 128], F16, tag="wq2")
            nc.gpsimd.dma_start(out=wq2, in_=wq2_d[:, :])
            wk2 = consts.tile([128, 128], F16, tag="wk2")
            nc.gpsimd.dma_start(out=wk2, in_=wk2_d[:, :])
            wv = consts.tile([128, D], F16, tag="wv")
            nc.gpsimd.dma_start(out=wv, in_=wv_d[:, :])

            nbias = consts.tile([128, 1], F32, tag="nbias")
            nc.vector.memset(nbias, -8.0)

            # ---- warmups: ACT table load + PE HAM ramp ----
            wsc = scratch_p.tile([128, 128], F32, tag="wexp")
            nc.scalar.activation(out=wsc, in_=wq2, func=mybir.ActivationFunctionType.Exp, scale=0.01)
            for _ in range(8):
                wps = pr_ps.tile([128, 128], F32, tag="pro", name="wps")
                nc.tensor.matmul(out=wps, lhsT=ident, rhs=ident, start=True, stop=True)

            state = {}

            def make_prologue(b):
                xt = xt_p.tile([128, T], F16, tag="xt", name="xt")
                for dj in range(NQ):
                    nc.sync.dma_start(
                        out=xt[:, QB * dj : QB * (dj + 1)],
                        in_=xt_d[b, :, QB * dj : QB * (dj + 1)],
                    )
                # Q/K projections (weights duplicated -> output rows 0:64 and 64:128)
                qt = qt_p.tile([128, T], F16, tag="qt", name="qt")
                kt = kt_p.tile([128, T], F16, tag="kt", name="kt")
                v1 = v1_p.tile([128, NCHUNK, D + 1], F16, tag="v1", name="v1")
                nc.vector.memset(v1[:, :, D : D + 1], 1.0)
                state[b] = (qt, kt, v1)

                def proj_qk(j):
                    pq = pr_ps.tile([128, QB], F32, tag="pro", name="pq")
                    hq = 64 * (j % 2)
                    nc.tensor.matmul(out=pq, lhsT=wq2[hq : hq + 64, :], rhs=xt[hq : hq + 64, QB * j : QB * (j + 1)], start=True, stop=True)
                    nc.vector.tensor_copy(out=qt[:, QB * j : QB * (j + 1)], in_=pq)
                    pk = pr_ps.tile([128, QB], F32, tag="pro", name="pk")
                    nc.tensor.matmul(out=pk, lhsT=wk2[64 - hq : 128 - hq, :], rhs=xt[64 - hq : 128 - hq, QB * j : QB * (j + 1)], start=True, stop=True)
                    nc.vector.tensor_copy(out=kt[:, QB * j : QB * (j + 1)], in_=pk)

                def proj_v(g):
                    pvp = pr_ps.tile([128, QB], F32, tag="pro", name="pvp")
                    for k in range(8):
                        t = 8 * g + k
                        nc.tensor.matmul(
                            out=pvp[:, D * k : D * (k + 1)],
                            lhsT=xt[0:64, 128 * t : 128 * (t + 1)],
                            rhs=wv[0:64, :],
                            start=True,
                            stop=True,
                        )
                    nc.vector.tensor_copy(
                        out=v1[:, 8 * g : 8 * (g + 1), 0:D],
                        in_=pvp.rearrange("p (k c) -> p k c", c=D),
                    )

                return proj_qk, proj_v

            TPC = 2  # chunks per ST/PT tile (ACT granularity = 1024 cols)

            def main_superblock(b, m):
                qt, kt, v1 = state[b]
                nch = 4 * m + 4
                ntiles = (nch + TPC - 1) // TPC
                yt1 = yt_ps.tile([128, QB], F32, tag="yt1")
                st_tiles = []
                pt_tiles = []

                def emit_st(c):
                    ti, slot = divmod(c, TPC)
                    if slot == 0:
                        st_tiles.append(st_ps.tile([128, QB * TPC], F32, tag="st", name="st_t"))
                    st_t = st_tiles[ti]
                    j = c - 4 * m
                    qoff = 128 * j if j >= 0 else 0
                    half = 64 * (c % 2)
                    nc.tensor.matmul(
                        out=st_t[:, QB * slot + qoff : QB * (slot + 1)],
                        lhsT=kt[half : half + 64, KB * c : KB * (c + 1)],
                        rhs=qt[half : half + 64, QB * m + qoff : QB * (m + 1)],
                        start=True,
                        stop=True,
                    )

                def emit_act_pv(ti):
                    st_t = st_tiles[ti]
                    c0 = TPC * ti
                    c1 = min(c0 + TPC, nch)
                    width = QB * (c1 - c0)
                    pt = pt_p.tile([128, QB * TPC], F16, tag="pt", name="pt")
                    pt_tiles.append(pt)
                    garbage = sum(
                        128 * (c - 4 * m) for c in range(c0, c1) if c - 4 * m > 0
                    )
                    if garbage >= 384:
                        # skip exp over never-read columns of diagonal chunks
                        for c in range(c0, c1):
                            j = c - 4 * m
                            qoff = 128 * j if j >= 0 else 0
                            slot = c - c0
                            nc.scalar.activation(
                                out=pt[:, QB * slot + qoff : QB * (slot + 1)],
                                in_=st_t[:, QB * slot + qoff : QB * (slot + 1)],
                                func=mybir.ActivationFunctionType.Exp,
                                bias=nbias,
                                scale=0.125,
                            )
                    else:
                        nc.scalar.activation(
                            out=pt[:, :width],
                            in_=st_t[:, :width],
                            func=mybir.ActivationFunctionType.Exp,
                            bias=nbias,
                            scale=0.125,
                        )
                    for c in range(c0, c1):
                        j = c - 4 * m
                        if j >= 0:
                            slot = c - c0
                            sub = pt[:, QB * slot + 128 * j : QB * slot + 128 * (j + 1)]
                            nc.gpsimd.affine_select(
                                out=sub,
                                in_=sub,
                                compare_op=mybir.AluOpType.is_ge,
                                fill=0.0,
                                base=0,
                                pattern=[[1, 128]],
                                channel_multiplier=-1,
                            )
                    for c in range(c0, c1):
                        j = c - 4 * m
                        qoff = 128 * j if j >= 0 else 0
                        slot = c - c0
                        nc.tensor.matmul(
                            out=yt1[0 : D + 1, qoff:QB],
                            lhsT=v1[:, c, :],
                            rhs=pt[:, QB * slot + qoff : QB * (slot + 1)],
                            start=(c == 0),
                            stop=(c == nch - 1),
                            skip_group_check=True,
                        )

                for c in range(min(TPC, nch)):
                    emit_st(c)
                for ti in range(1, ntiles):
                    for c in range(TPC * ti, min(TPC * (ti + 1), nch)):
                        emit_st(c)
                    emit_act_pv(ti - 1)
                emit_act_pv(ntiles - 1)
                ytsb = scratch_p.tile([D + 1, QB], F32, tag="ytsb", name="ytsb")
                nc.vector.tensor_copy(out=ytsb, in_=yt1[0 : D + 1, :])
                nc.sync.dma_start(out=yt1_d[b, m, :, :], in_=ytsb)

            # emission: projections interleaved just-in-time between
            # superblocks so ACT never starves; b1 prologue spread over
            # b0's large tail superblocks.
            pq0, pv0 = make_prologue(0)
            pq0(0); pv0(0)
            main_superblock(0, 0)
            pq0(1); main_superblock(0, 1)
            pq0(2); pv0(1); main_superblock(0, 2)
            pq0(3); main_superblock(0, 3)
            pq0(4); pv0(2); main_superblock(0, 4)
            pq1, pv1 = make_prologue(1)
            pq0(5); pv0(3); main_superblock(0, 5)
            pq0(6); pq1(0); pv1(0); main_superblock(0, 6)
            pq0(7); pq1(1); pq1(2); pv1(1); main_superblock(0, 7)
            pq1(3); pq1(4); pv1(2); main_superblock(1, 0)
            pq1(5); pq1(6); pv1(3); main_superblock(1, 1)
            pq1(7); main_superblock(1, 2)
            for m in range(3, NQ):
                main_superblock(1, m)

    nc.finalize()
    return nc


_NC = None


def _get_nc():
    global _NC
    if _NC is None:
        _NC = _build()
    return _NC


def _run(x, Wk, Wq, Wv, trace=False):
    x = np.ascontiguousarray(np.asarray(x, dtype=np.float32))
    Wk = np.asarray(Wk, dtype=np.float32)
    Wq = np.asarray(Wq, dtype=np.float32)
    Wv = np.asarray(Wv, dtype=np.float32)
    B = x.shape[0]
    assert B == N_CORES * B_LOC and x.shape[1] == T and x.shape[2] == D

    wq2 = np.concatenate([Wq.T, Wq.T], axis=1)
    wq2 = np.ascontiguousarray(np.concatenate([wq2, wq2], axis=0))
    wk2 = np.concatenate([Wk.T, Wk.T], axis=1)
    wk2 = np.ascontiguousarray(np.concatenate([wk2, wk2], axis=0))
    wv = np.ascontiguousarray(np.concatenate([Wv.T, Wv.T], axis=0))
    ident = np.eye(128, dtype=np.float32)

    xt16 = x.astype(np.float16).transpose(0, 2, 1)
    xt16 = np.ascontiguousarray(np.concatenate([xt16, xt16], axis=1))
    in_maps = []
    for c in range(N_CORES):
        in_maps.append(
            {
                "xt16": np.ascontiguousarray(xt16[B_LOC * c : B_LOC * (c + 1)]),
                "wq2": wq2,
                "wk2": wk2,
                "wv": wv,
                "ident": ident,
            }
        )

    nc = _get_nc()
    res = run_bass_kernel_spmd(nc, in_maps, core_ids=list(range(N_CORES)), trace=trace)

    y = np.empty((B, T, D), dtype=np.float32)
    for c in range(N_CORES):
        yt1 = res.results[c]["yt1"]  # [B_LOC, NQ, 65, 512]
        num = yt1[:, :, :D, :]
        den = yt1[:, :, D : D + 1, :]
        yb = (num / den).transpose(0, 1, 3, 2).reshape(B_LOC, T, D)
        y[B_LOC * c : B_LOC * (c + 1)] = yb
    return y, res


def kernel(x, Wk, Wq, Wv):
    y, _ = _run(x, Wk, Wq, Wv, trace=False)
    return y


# revision 23
# speedup vs baseline: 1.2748x; 1.0572x over previous
"""Single-head causal self-attention on 8 trn2 NeuronCores.

B=16, T=4096, D=64 fp32. Data-parallel over batch: 2 batches per core.
Per core/batch: x -> xT (PE transpose), QT/KT (dup'd weight proj), V1=[V|1]
natural layout; scores computed transposed (keys on partitions) in f32r,
exp on ScalarE with fused 1/sqrt(D) scale, causal via chunk skipping +
N-restriction + affine_select on diagonal blocks; PV matmul accumulates
[Y^T; denom] in PSUM via ones-column; normalization happens on host.
"""
import os
import sys

os.environ.setdefault("MYCRO_LOCAL_CACHE", "1")
sys.path.insert(0, "/opt/trn_rl_repo")

import numpy as np

import concourse.bass as bass
import concourse.tile as tile
from concourse import bacc, mybir
from concourse.bass_utils import run_bass_kernel_spmd

F32 = mybir.dt.float32
F32R = mybir.dt.float32r
F16 = mybir.dt.float16

N_CORES = 8
B_LOC = 2          # batches per core
T = 4096
D = 64
NQ = 8             # q-superblocks of 512 per batch
QB = 512           # q-superblock width
KB = 128           # key chunk (PSUM partition dim of scores)
NCHUNK = T // KB   # 32 key chunks per batch


def _build():
    nc = bacc.Bacc(None)

    xt_d = nc.declare_dram_parameter("xt16", [B_LOC, 128, T], F16, isOutput=False)
    wq2_d = nc.declare_dram_parameter("wq2", [128, 128], F32, isOutput=False)
    wk2_d = nc.declare_dram_parameter("wk2", [128, 128], F32, isOutput=False)
    wv_d = nc.declare_dram_parameter("wv", [128, D], F32, isOutput=False)
    id_d = nc.declare_dram_parameter("ident", [128, 128], F32, isOutput=False)
    yt1_d = nc.declare_dram_parameter("yt1", [B_LOC, NQ, D + 1, QB], F32, isOutput=True)

    with tile.TileContext(nc) as tc:
        with (
            tc.tile_pool(name="consts", bufs=1) as consts,
            tc.tile_pool(name="xt", bufs=2) as xt_p,
            tc.tile_pool(name="qt", bufs=2) as qt_p,
            tc.tile_pool(name="kt", bufs=2) as kt_p,
            tc.tile_pool(name="v1", bufs=2) as v1_p,
            tc.tile_pool(name="pt", bufs=3) as pt_p,
            tc.tile_pool(name="scratch", bufs=2) as scratch_p,
            tc.tile_pool(name="stps", bufs=3, space="PSUM") as st_ps,
            tc.tile_pool(name="ytps", bufs=2, space="PSUM") as yt_ps,
        ):
            # ---- constants ----
            ident = consts.tile([128, 128], F16, tag="ident")
            nc.gpsimd.dma_start(out=ident, in_=id_d[:, :])
            wq2 = consts.tile([128, 128], F16, tag="wq2")
            nc.gpsimd.dma_start(out=wq2, in_=wq2_d[:, :])
            wk2 = consts.tile([128, 128], F16, tag="wk2")
            nc.gpsimd.dma_start(out=wk2, in_=wk2_d[:, :])
            wv = consts.tile([128, D], F16, tag="wv")
            nc.gpsimd.dma_start(out=wv, in_=wv_d[:, :])

            nbias = consts.tile([128, 1], F32, tag="nbias")
            nc.vector.memset(nbias, -8.0)

            # ---- warmups: ACT table load + PE HAM ramp ----
            wsc = scratch_p.tile([128, 128], F32, tag="wexp")
            nc.scalar.activation(out=wsc, in_=wq2, func=mybir.ActivationFunctionType.Exp, scale=0.01)
            for _ in range(8):
                wps = yt_ps.tile([128, 128], F32, tag="yt1", name="wps")
                nc.tensor.matmul(out=wps, lhsT=ident, rhs=ident, start=True, stop=True)

            state = {}

            def make_prologue(b):
                xt = xt_p.tile([128, T], F16, tag="xt", name="xt")
                for dj in range(NQ):
                    nc.sync.dma_start(
                        out=xt[:, QB * dj : QB * (dj + 1)],
                        in_=xt_d[b, :, QB * dj : QB * (dj + 1)],
                    )
                # Q/K projections (weights duplicated -> output rows 0:64 and 64:128)
                qt = qt_p.tile([128, T], F16, tag="qt", name="qt")
                kt = kt_p.tile([128, T], F16, tag="kt", name="kt")
                v1 = v1_p.tile([128, NCHUNK, D + 1], F16, tag="v1", name="v1")
                nc.vector.memset(v1[:, :, D : D + 1], 1.0)
                state[b] = (qt, kt, v1)

                def proj_qk(j):
                    pq = yt_ps.tile([128, QB], F32, tag="yt1", name="pq")
                    hq = 64 * (j % 2)
                    nc.tensor.matmul(out=pq, lhsT=wq2[hq : hq + 64, :], rhs=xt[hq : hq + 64, QB * j : QB * (j + 1)], start=True, stop=True)
                    nc.vector.tensor_copy(out=qt[:, QB * j : QB * (j + 1)], in_=pq)
                    pk = yt_ps.tile([128, QB], F32, tag="yt1", name="pk")
                    nc.tensor.matmul(out=pk, lhsT=wk2[64 - hq : 128 - hq, :], rhs=xt[64 - hq : 128 - hq, QB * j : QB * (j + 1)], start=True, stop=True)
                    nc.vector.tensor_copy(out=kt[:, QB * j : QB * (j + 1)], in_=pk)

                def proj_v(g):
                    pvp = yt_ps.tile([128, QB], F32, tag="yt1", name="pvp")
                    for k in range(8):
                        t = 8 * g + k
                        nc.tensor.matmul(
                            out=pvp[:, D * k : D * (k + 1)],
                            lhsT=xt[0:64, 128 * t : 128 * (t + 1)],
                            rhs=wv[0:64, :],
                            start=True,
                            stop=True,
                        )
                    nc.vector.tensor_copy(
                        out=v1[:, 8 * g : 8 * (g + 1), 0:D],
                        in_=pvp.rearrange("p (k c) -> p k c", c=D),
                    )

                return proj_qk, proj_v

            TPC = 2  # chunks per ST/PT tile (ACT granularity = 1024 cols)

            def main_superblock(b, m):
                qt, kt, v1 = state[b]
                nch = 4 * m + 4
                ntiles = (nch + TPC - 1) // TPC
                yt1 = yt_ps.tile([128, QB], F32, tag="yt1")
                st_tiles = []
                pt_tiles = []

                def emit_st(c):
                    ti, slot = divmod(c, TPC)
                    if slot == 0:
                        st_tiles.append(st_ps.tile([128, QB * TPC], F32, tag="st", name="st_t"))
                    st_t = st_tiles[ti]
                    j = c - 4 * m
                    qoff = 128 * j if j >= 0 else 0
                    half = 64 * (c % 2)
                    nc.tensor.matmul(
                        out=st_t[:, QB * slot + qoff : QB * (slot + 1)],
                        lhsT=kt[half : half + 64, KB * c : KB * (c + 1)],
                        rhs=qt[half : half + 64, QB * m + qoff : QB * (m + 1)],
                        start=True,
                        stop=True,
                    )

                def emit_act_pv(ti):
                    st_t = st_tiles[ti]
                    c0 = TPC * ti
                    c1 = min(c0 + TPC, nch)
                    width = QB * (c1 - c0)
                    pt = pt_p.tile([128, QB * TPC], F16, tag="pt", name="pt")
                    pt_tiles.append(pt)
                    garbage = sum(
                        128 * (c - 4 * m) for c in range(c0, c1) if c - 4 * m > 0
                    )
                    if garbage >= 384:
                        # skip exp over never-read columns of diagonal chunks
                        for c in range(c0, c1):
                            j = c - 4 * m
                            qoff = 128 * j if j >= 0 else 0
                            slot = c - c0
                            nc.scalar.activation(
                                out=pt[:, QB * slot + qoff : QB * (slot + 1)],
                                in_=st_t[:, QB * slot + qoff : QB * (slot + 1)],
                                func=mybir.ActivationFunctionType.Exp,
                                bias=nbias,
                                scale=0.125,
                            )
                    else:
                        nc.scalar.activation(
                            out=pt[:, :width],
                            in_=st_t[:, :width],
                            func=mybir.ActivationFunctionType.Exp,
                            bias=nbias,
                            scale=0.125,
                        )
                    for c in range(c0, c1):
                        j = c - 4 * m
                        if j >= 0:
                            slot = c - c0
                            sub = pt[:, QB * slot + 128 * j : QB * slot + 128 * (j + 1)]
                            nc.gpsimd.affine_select(
                                out=sub,
                                in_=sub,
                                compare_op=mybir.AluOpType.is_ge,
                                fill=0.0,
                                base=0,
                                pattern=[[1, 128]],
                                channel_multiplier=-1,
                            )
                    for c in range(c0, c1):
                        j = c - 4 * m
                        qoff = 128 * j if j >= 0 else 0
                        slot = c - c0
                        nc.tensor.matmul(
                            out=yt1[0 : D + 1, qoff:QB],
                            lhsT=v1[:, c, :],
                            rhs=pt[:, QB * slot + qoff : QB * (slot + 1)],
                            start=(c == 0),
                            stop=(c == nch - 1),
                            skip_group_check=True,
                        )

                for c in range(min(TPC, nch)):
                    emit_st(c)
                for ti in range(1, ntiles):
                    for c in range(TPC * ti, min(TPC * (ti + 1), nch)):
                        emit_st(c)
                    emit_act_pv(ti - 1)
                emit_act_pv(ntiles - 1)
                ytsb = scratch_p.tile([D + 1, QB], F32, tag="ytsb", name="ytsb")
                nc.vector.tensor_copy(out=ytsb, in_=yt1[0 : D + 1, :])
                nc.sync.dma_start(out=yt1_d[b, m, :, :], in_=ytsb)

            # emission: projections interleaved just-in-time between
            # superblocks so ACT never starves; b1 prologue spread over
            # b0's large tail superblocks.
            pq0, pv0 = make_prologue(0)
            pq0(0); pv0(0)
            main_superblock(0, 0)
            pq0(1); main_superblock(0, 1)
            pq0(2); pv0(1); main_superblock(0, 2)
            pq0(3); main_superblock(0, 3)
            pq0(4); pv0(2); main_superblock(0, 4)
            pq1, pv1 = make_prologue(1)
            pq0(5); pv0(3); main_superblock(0, 5)
            pq0(6); pq1(0); pv1(0); main_superblock(0, 6)
            pq0(7); pq1(1); pq1(2); pv1(1); main_superblock(0, 7)
            pq1(3); pq1(4); pv1(2); main_superblock(1, 0)
            pq1(5); pq1(6); pv1(3); main_superblock(1, 1)
            pq1(7); main_superblock(1, 2)
            for m in range(3, NQ):
                main_superblock(1, m)

    nc.finalize()
    return nc


_NC = None


def _get_nc():
    global _NC
    if _NC is None:
        _NC = _build()
    return _NC


def _run(x, Wk, Wq, Wv, trace=False):
    x = np.ascontiguousarray(np.asarray(x, dtype=np.float32))
    Wk = np.asarray(Wk, dtype=np.float32)
    Wq = np.asarray(Wq, dtype=np.float32)
    Wv = np.asarray(Wv, dtype=np.float32)
    B = x.shape[0]
    assert B == N_CORES * B_LOC and x.shape[1] == T and x.shape[2] == D

    wq2 = np.concatenate([Wq.T, Wq.T], axis=1)
    wq2 = np.ascontiguousarray(np.concatenate([wq2, wq2], axis=0))
    wk2 = np.concatenate([Wk.T, Wk.T], axis=1)
    wk2 = np.ascontiguousarray(np.concatenate([wk2, wk2], axis=0))
    wv = np.ascontiguousarray(np.concatenate([Wv.T, Wv.T], axis=0))
    ident = np.eye(128, dtype=np.float32)

    xt16 = x.astype(np.float16).transpose(0, 2, 1)
    xt16 = np.ascontiguousarray(np.concatenate([xt16, xt16], axis=1))
    in_maps = []
    for c in range(N_CORES):
        in_maps.append(
            {
                "xt16": np.ascontiguousarray(xt16[B_LOC * c : B_LOC * (c + 1)]),
                "wq2": wq2,
                "wk2": wk2,
                "wv": wv,
                "ident": ident,
            }
        )

    nc = _get_nc()
    res = run_bass_kernel_spmd(nc, in_maps, core_ids=list(range(N_CORES)), trace=trace)

    y = np.empty((B, T, D), dtype=np.float32)
    for c in range(N_CORES):
        yt1 = res.results[c]["yt1"]  # [B_LOC, NQ, 65, 512]
        num = yt1[:, :, :D, :]
        den = yt1[:, :, D : D + 1, :]
        yb = (num / den).transpose(0, 1, 3, 2).reshape(B_LOC, T, D)
        y[B_LOC * c : B_LOC * (c + 1)] = yb
    return y, res


def kernel(x, Wk, Wq, Wv):
    y, _ = _run(x, Wk, Wq, Wv, trace=False)
    return y
